# revision 8
# baseline (speedup 1.0000x reference)
"""3-layer GAT on 8 trn2 NeuronCores.

Strategy
--------
Nodes are permuted (per-core in-degree sort) and dealt to 8 cores (edge-
balanced); each core owns a contiguous range of 6272 new ids (6250 real +
pad).  One Bass program runs 3 times (one launch per GAT layer); the host
concatenates per-core outputs between launches (pure data movement).

Per launch, each core:
  1. BN-affine + relu-switch on the full feature matrix XT [128, 50176]
     (feature-major, bf16), then matmul -> h table T [50176, 128] bf16 in
     DRAM (replicated per core).  Layer-0 BN is an exact identity via
     g = sqrt(eps), partials = 0.
  2. For each of 49 dst blocks (128 dsts each, ELL format): bulk-gather
     the block's edge source rows from T via dma_gather (int16 indices;
     two overlapping table windows [0,32K) and [N-32K,N) dodge the int16
     range limit), compute attention logits (al_s = G . a_src on DVE,
     al_d from the self-loop slot), masked LeakyReLU+exp softmax with
     per-dst denominators, weighted-sum aggregation, head-mix matmul
     (identity for layers 0/1, head-mean for layer 2), bias.
  3. Emits bf16 feature-major output (next layer's XT shard), fp32 copy
     (graded output for layer 2), and BN partial sums.
"""
import os
import numpy as np

import concourse.bass as bass
import concourse.bacc as bacc
import concourse.mybir as mybir
import concourse.tile as tile
from concourse import bass_utils
from concourse.masks import make_identity
from concourse.tile_sem_assignment import PROC_NAME_TO_IDX

_IDX_TO_PROC = {v: k for k, v in PROC_NAME_TO_IDX.items()}

def _bc(ap, pos, count):
    """Insert a step-0 (broadcast) axis into an AP at position pos."""
    lst = [list(x) for x in ap.ap]
    lst.insert(pos, [0, count])
    return bass.AP(ap.tensor, ap.offset, lst)


F32 = mybir.dt.float32
BF16 = mybir.dt.bfloat16
I16 = mybir.dt.int16

N = 50000
E = 800000
H = 2
CH = 64
IN = 128
OUT = 64
EPS = 1e-5
SLOPE = 0.2
NEG = -30000.0

N_CORES = 8
PER_CORE = 6272            # 49 * 128
NPAD = N_CORES * PER_CORE  # 50176
NBLK = PER_CORE // 128     # 49
REAL_PER_CORE = N // N_CORES  # 6250
D = 128                    # table row elems (bf16) = 256B
LO_END = 32768             # lo window [0, LO_END)
HI_START = NPAD - 32768    # hi window [HI_START, NPAD)
NQ = 4


# ----------------------------------------------------------------- host prep

def _wrap_idxs(flat):
    """flat [n] int -> dma_gather idx layout [128, n/16] int16 (wrapped in 16
    partitions, i = s*16 + p, replicated across the 8 q7 core groups)."""
    n = flat.shape[0]
    w = flat.reshape(n // 16, 16).T.astype(np.int16)
    return np.tile(w, (8, 1))


def preprocess(edge_index):
    """Build node permutation, per-core ELL grids and masks."""
    src = edge_index[0].astype(np.int64)
    dst = edge_index[1].astype(np.int64)

    indeg = np.bincount(dst, minlength=N) + 1  # + self loop
    # deal nodes to cores, balancing edges: sort by in-degree, snake-deal
    order = np.argsort(-indeg, kind="stable")
    core_of = np.empty(N, np.int32)
    for i in range(N):
        r = i % (2 * N_CORES)
        core_of[order[i]] = r if r < N_CORES else 2 * N_CORES - 1 - r

    # strict-lo counts per dst (needed for the block sub-sort): computed
    # against the *new* numbering, which depends on the permutation itself.
    # Two passes: first assign ranks by (core, -indeg), then compute window
    # classes, then refine the within-core order by (-indeg, strict_lo).
    def ranks_for(key_strict):
        new_id = np.empty(N, np.int64)
        for c in range(N_CORES):
            nodes = np.where(core_of == c)[0]
            if key_strict is None:
                k = np.lexsort((nodes, -indeg[nodes]))
            else:
                k = np.lexsort((nodes, key_strict[nodes], -indeg[nodes]))
            new_id[nodes[k]] = c * PER_CORE + np.arange(len(nodes))
        return new_id

    new_id = ranks_for(None)
    # strict-lo for each dst under this tentative numbering
    s_new = new_id[src]
    strict_lo_per_dst = np.bincount(
        dst, weights=(s_new < HI_START).astype(np.float64), minlength=N
    )
    new_id = ranks_for(strict_lo_per_dst)

    ns = new_id[src]
    nd = new_id[dst]
    # self loops
    self_ids = new_id[np.arange(N)]
    ns = np.concatenate([ns, self_ids])
    nd = np.concatenate([nd, self_ids])

    # group edges by destination
    o = np.argsort(nd, kind="stable")
    ns, nd = ns[o], nd[o]
    starts = np.searchsorted(nd, np.arange(NPAD))
    ends = np.searchsorted(nd, np.arange(NPAD) + 1)

    # per (core, block) window budgeting
    KLO = np.zeros(NBLK, np.int64)
    KHI = np.zeros(NBLK, np.int64)
    lists = {}  # (c, b) -> list of per-row (lo_srcs, hi_srcs)
    for c in range(N_CORES):
        base_c = c * PER_CORE
        for b in range(NBLK):
            rows = []
            lo_block = (base_c + b * 128 + 128) <= LO_END
            for p in range(128):
                d_node = base_c + b * 128 + p
                sl = ns[starts[d_node]:ends[d_node]]
                is_self = sl == d_node
                # exactly one self slot pinned to column 0 of the block's
                # window regime; duplicate (d,d) edges stay as regular slots
                s_self = sl[is_self][:1]
                extra_self = sl[is_self][1:]
                s_rest = np.concatenate([sl[~is_self], extra_self])
                must_lo = s_rest[s_rest < HI_START]
                must_hi = s_rest[s_rest >= LO_END]
                mid = s_rest[(s_rest >= HI_START) & (s_rest < LO_END)]
                rows.append((lo_block, s_self, must_lo, must_hi, mid))
            # feasibility: klo >= max(must_lo + self_lo), khi >= ...
            klo_min = max(
                len(r[2]) + (len(r[1]) if r[0] else 0) for r in rows
            )
            khi_min = max(
                len(r[3]) + (0 if r[0] else len(r[1])) for r in rows
            )
            deg_max = max(len(r[1]) + len(r[2]) + len(r[3]) + len(r[4]) for r in rows)
            klo = klo_min
            khi = max(khi_min, deg_max - klo)
            # balance: grow klo if it reduces total (it can't: total >= deg_max)
            KLO[b] = max(KLO[b], klo)
            KHI[b] = max(KHI[b], khi)
            lists[(c, b)] = rows

    tot_slots = int((KLO + KHI).sum() * 128)
    tot_edges = len(ns) // N_CORES
    print(f"[prep] slots/core {tot_slots} vs edges/core ~{tot_edges} "
          f"(pad {tot_slots / tot_edges - 1:.1%})  K={int((KLO + KHI).sum())}")

    grids_lo = np.zeros((N_CORES, 128, int(KLO.sum())), np.int64)
    grids_hi = np.zeros((N_CORES, 128, int(KHI.sum())), np.int64)
    gmask = np.full((N_CORES, 128, int((KLO + KHI).sum())), NEG, np.float32)
    mself = np.zeros((N_CORES, 128, NBLK), np.float32)
    olo = np.concatenate([[0], np.cumsum(KLO)])
    ohi = np.concatenate([[0], np.cumsum(KHI)])
    ok = np.concatenate([[0], np.cumsum(KLO + KHI)])
    for c in range(N_CORES):
        for b in range(NBLK):
            klo, khi = int(KLO[b]), int(KHI[b])
            for p in range(128):
                lo_block, s_self, must_lo, must_hi, mid = lists[(c, b)][p]
                lo = list(must_lo)
                hi = list(must_hi)
                if len(s_self):
                    (lo if lo_block else hi).insert(0, s_self[0])
                # distribute mid: fill lo up to klo, rest to hi
                mid = list(mid)
                room_lo = klo - len(lo)
                lo += mid[:room_lo]
                hi += mid[room_lo:]
                assert len(lo) <= klo and len(hi) <= khi, (c, b, p, len(lo), len(hi))
                g = grids_lo[c, p]
                g[olo[b]:olo[b] + len(lo)] = lo
                g[olo[b] + len(lo):olo[b + 1]] = lo[0] if lo else 0
                g2 = grids_hi[c, p]
                g2[ohi[b]:ohi[b] + len(hi)] = hi
                g2[ohi[b] + len(hi):ohi[b + 1]] = hi[0] if hi else HI_START
                m = gmask[c, p]
                m[ok[b]:ok[b] + len(lo)] = 0.0
                m[ok[b] + klo:ok[b] + klo + len(hi)] = 0.0
                mself[c, p, b] = 1.0 if lo_block else 0.0

    # wrap grids for dma_gather (per block, lo and hi separately)
    glo_w = np.zeros((N_CORES, 128, 8 * int(KLO.sum())), np.int16)
    ghi_w = np.zeros((N_CORES, 128, 8 * int(KHI.sum())), np.int16)
    for c in range(N_CORES):
        for b in range(NBLK):
            klo, khi = int(KLO[b]), int(KHI[b])
            fl = grids_lo[c, :, olo[b]:olo[b + 1]].T.reshape(-1)  # (j, p) order
            glo_w[c, :, 8 * olo[b]:8 * olo[b + 1]] = _wrap_idxs(fl)
            fh = grids_hi[c, :, ohi[b]:ohi[b + 1]].T.reshape(-1) - HI_START
            ghi_w[c, :, 8 * ohi[b]:8 * ohi[b + 1]] = _wrap_idxs(fh)

    return dict(new_id=new_id, KLO=KLO.tolist(), KHI=KHI.tolist(),
                glo=glo_w, ghi=ghi_w, gmask=gmask, mself=mself)


# ----------------------------------------------------------------- builder

def build(KLO, KHI):
    nc = bacc.Bacc(None, target_bir_lowering=False, debug=False,
                   num_devices=N_CORES, num_swdge_queues=NQ)
    KSUM = [a + b for a, b in zip(KLO, KHI)]
    SLO, SHI, SK = sum(KLO), sum(KHI), sum(KSUM)

    xt = nc.dram_tensor("xt", [128, NPAD], BF16, kind="ExternalInput")
    part = nc.dram_tensor("part", [128, 16], F32, kind="ExternalInput")
    gvec = nc.dram_tensor("gvec", [128, 1], F32, kind="ExternalInput")
    bevec = nc.dram_tensor("bevec", [128, 1], F32, kind="ExternalInput")
    srel = nc.dram_tensor("srel", [128, 1], F32, kind="ExternalInput")
    wmat = nc.dram_tensor("wmat", [128, 128], BF16, kind="ExternalInput")
    asrct = nc.dram_tensor("asrct", [128, 128], BF16, kind="ExternalInput")
    adstt = nc.dram_tensor("adstt", [128, 128], BF16, kind="ExternalInput")
    mmat = nc.dram_tensor("mmat", [128, 128], F32, kind="ExternalInput")
    biasv = nc.dram_tensor("biasv", [128, 1], F32, kind="ExternalInput")
    maskf = nc.dram_tensor("maskf", [128, 128], F32, kind="ExternalInput")
    mselfd = nc.dram_tensor("mself", [128, NBLK], F32, kind="ExternalInput")
    glod = nc.dram_tensor("glo", [128, 8 * SLO], I16, kind="ExternalInput")
    ghid = nc.dram_tensor("ghi", [128, 8 * SHI], I16, kind="ExternalInput")
    gmaskd = nc.dram_tensor("gmask", [128, SK], F32, kind="ExternalInput")

    outb = nc.dram_tensor("outb", [128, PER_CORE], BF16, kind="ExternalOutput")
    outf = nc.dram_tensor("outf", [128, PER_CORE], F32, kind="ExternalOutput")
    parts = nc.dram_tensor("parts", [128, 2], F32, kind="ExternalOutput")

    tbl = nc.dram_tensor("tbl", [NPAD, D], BF16)  # internal

    with tile.TileContext(nc) as tc:
        with (
            tc.tile_pool(name="const", bufs=1) as cpool,
            tc.tile_pool(name="norm", bufs=3) as npool,
            tc.tile_pool(name="tw", bufs=4) as twpool,
            tc.tile_pool(name="grid", bufs=3) as grpool,
            tc.tile_pool(name="g", bufs=2) as gpool,
            tc.tile_pool(name="work", bufs=3) as wpool,
            tc.tile_pool(name="small", bufs=4) as spool,
            tc.tile_pool(name="acc", bufs=1) as apool,
            tc.tile_pool(name="ps", bufs=2, space="PSUM") as pspool,
            tc.tile_pool(name="ps2", bufs=2, space="PSUM") as ps2pool,
        ):
            ident = cpool.tile([128, 128], F32, tag="ident")
            make_identity(nc, ident[:])

            # --- BN params ------------------------------------------------
            pt = cpool.tile([128, 16], F32, tag="pt")
            nc.sync.dma_start(pt[:], part.ap())
            gv = cpool.tile([128, 1], F32, tag="gv")
            nc.sync.dma_start(gv[:], gvec.ap())
            bev = cpool.tile([128, 1], F32, tag="bev")
            nc.sync.dma_start(bev[:], bevec.ap())
            sv = cpool.tile([128, 1], F32, tag="sv")
            nc.sync.dma_start(sv[:], srel.ap())

            s1 = cpool.tile([128, 1], F32, tag="s1")
            s2 = cpool.tile([128, 1], F32, tag="s2")
            nc.vector.reduce_sum(s1[:], pt[:, 0:8], axis=mybir.AxisListType.X)
            nc.vector.reduce_sum(s2[:], pt[:, 8:16], axis=mybir.AxisListType.X)
            mu = cpool.tile([128, 1], F32, tag="mu")
            nc.vector.tensor_scalar_mul(mu[:], s1[:], 1.0 / N)
            msq = cpool.tile([128, 1], F32, tag="msq")
            nc.vector.tensor_scalar_mul(msq[:], s2[:], 1.0 / N)
            var = cpool.tile([128, 1], F32, tag="var")
            nc.vector.tensor_tensor(out=var[:], in0=mu[:], in1=mu[:],
                                    op=mybir.AluOpType.mult)
            nc.vector.tensor_tensor(out=var[:], in0=msq[:], in1=var[:],
                                    op=mybir.AluOpType.subtract)
            sd = cpool.tile([128, 1], F32, tag="sd")
            epsT = cpool.tile([128, 1], F32, tag="epsT")
            nc.vector.memset(epsT[:], EPS)
            nc.scalar.activation(sd[:], var[:], mybir.ActivationFunctionType.Sqrt,
                                 bias=epsT[:], scale=1.0)
            ra = cpool.tile([128, 1], F32, tag="ra")
            nc.vector.reciprocal(ra[:], sd[:])
            av = cpool.tile([128, 1], F32, tag="av")
            nc.vector.tensor_tensor(out=av[:], in0=ra[:], in1=gv[:],
                                    op=mybir.AluOpType.mult)
            bv = cpool.tile([128, 1], F32, tag="bv")
            nc.vector.tensor_tensor(out=bv[:], in0=mu[:], in1=av[:],
                                    op=mybir.AluOpType.mult)
            nc.vector.tensor_tensor(out=bv[:], in0=bev[:], in1=bv[:],
                                    op=mybir.AluOpType.subtract)

            wt = cpool.tile([128, 128], BF16, tag="wt")
            nc.sync.dma_start(wt[:], wmat.ap())
            asr = cpool.tile([128, 128], BF16, tag="asr")
            nc.sync.dma_start(asr[:], asrct.ap())
            ads = cpool.tile([128, 128], BF16, tag="ads")
            nc.sync.dma_start(ads[:], adstt.ap())
            mm = cpool.tile([128, 128], F32, tag="mm")
            nc.sync.dma_start(mm[:], mmat.ap())
            bi = cpool.tile([128, 1], F32, tag="bi")
            nc.sync.dma_start(bi[:], biasv.ap())
            mft = cpool.tile([128, 128], F32, tag="mft")
            nc.sync.dma_start(mft[:], maskf.ap())
            msf = cpool.tile([128, NBLK], F32, tag="msf")
            nc.sync.dma_start(msf[:], mselfd.ap())

            # --- table build: T[r] = relu_s(bn(x))^T @ W -------------------
            CH_N = 1024
            for r0 in range(0, NPAD, CH_N):
                xn = npool.tile([128, CH_N], BF16, tag="xn")
                nc.sync.dma_start(xn[:], xt.ap()[:, r0:r0 + CH_N])
                u = npool.tile([128, CH_N], BF16, tag="u")
                nc.vector.tensor_scalar(out=u[:], in0=xn[:], scalar1=av[:],
                                        scalar2=bv[:], op0=mybir.AluOpType.mult,
                                        op1=mybir.AluOpType.add)
                v = npool.tile([128, CH_N], BF16, tag="v")
                nc.vector.tensor_scalar(out=v[:], in0=u[:], scalar1=sv[:],
                                        scalar2=None, op0=mybir.AluOpType.mult)
                nc.vector.tensor_tensor(out=u[:], in0=u[:], in1=v[:],
                                        op=mybir.AluOpType.max)
                for rr in range(0, CH_N, 128):
                    hp = pspool.tile([128, 128], F32, tag="hp", space="PSUM")
                    nc.tensor.matmul(hp[:], lhsT=u[:, rr:rr + 128], rhs=wt[:],
                                     start=True, stop=True)
                    hb = twpool.tile([128, 128], BF16, tag="hb")
                    nc.vector.tensor_copy(hb[:], hp[:])
                    nc.sync.dma_start(tbl.ap()[r0 + rr:r0 + rr + 128, :], hb[:])

            # --- per-block aggregation ------------------------------------
            pacc = apool.tile([128, 2], F32, tag="pacc")
            nc.vector.memset(pacc[:], 0.0)
            olo = np.concatenate([[0], np.cumsum(KLO)]).astype(int)
            ohi = np.concatenate([[0], np.cumsum(KHI)]).astype(int)
            ok = np.concatenate([[0], np.cumsum(KSUM)]).astype(int)
            qn = 0
            for b in range(NBLK):
                klo, khi = KLO[b], KHI[b]
                k = klo + khi
                glt = grpool.tile([128, 8 * klo], I16, tag="glt")
                nc.sync.dma_start(glt[:], glod.ap()[:, 8 * olo[b]:8 * olo[b + 1]])
                ght = grpool.tile([128, 8 * khi], I16, tag="ght")
                nc.sync.dma_start(ght[:], ghid.ap()[:, 8 * ohi[b]:8 * ohi[b + 1]])
                mk = grpool.tile([128, k], F32, tag="mk")
                nc.sync.dma_start(mk[:], gmaskd.ap()[:, ok[b]:ok[b + 1]])

                g = gpool.tile([128, k * D], BF16, tag="g")
                g3 = g[:].rearrange("p (k d) -> p k d", d=D)
                nc.gpsimd.dma_gather(
                    out_ap=g3[:, 0:klo, :], in_ap=tbl.ap()[0:LO_END, :],
                    idxs_ap=glt[:], num_idxs=128 * klo, num_idxs_reg=128 * klo,
                    elem_size=D, single_packet=False, queue_num=qn % NQ)
                qn += 1
                nc.gpsimd.dma_gather(
                    out_ap=g3[:, klo:k, :], in_ap=tbl.ap()[HI_START:NPAD, :],
                    idxs_ap=ght[:], num_idxs=128 * khi, num_idxs_reg=128 * khi,
                    elem_size=D, single_packet=False, queue_num=qn % NQ)
                qn += 1

                # al_s[p, j, h] = sum_c G[p,j,hc] * a_src[h,c]
                tmp = wpool.tile([128, k * D], BF16, tag="tmp")
                nc.vector.tensor_tensor(
                    out=tmp[:].rearrange("p (k d) -> p k d", d=D),
                    in0=g3,
                    in1=_bc(asr[:], 1, k),
                    op=mybir.AluOpType.mult)
                als = spool.tile([128, k * H], F32, tag="als")
                nc.vector.reduce_sum(
                    als[:].rearrange("p (k h) -> p k h", h=H),
                    tmp[:].rearrange("p (k h c) -> p k h c", h=H, c=CH),
                    axis=mybir.AxisListType.X)

                # al_d candidates from self slots (col 0 and col klo)
                tmpd = spool.tile([128, 2 * D], BF16, tag="tmpd")
                selfsl = g3[:, 0:klo + 1:max(klo, 1), :]  # cols {0, klo}
                nc.vector.tensor_tensor(
                    out=tmpd[:].rearrange("p (t d) -> p t d", d=D),
                    in0=selfsl,
                    in1=_bc(ads[:], 1, 2),
                    op=mybir.AluOpType.mult)
                aldc = spool.tile([128, 2 * H], F32, tag="aldc")
                nc.vector.reduce_sum(
                    aldc[:].rearrange("p (t h) -> p t h", h=H),
                    tmpd[:].rearrange("p (t h c) -> p t h c", h=H, c=CH),
                    axis=mybir.AxisListType.X)
                ald = spool.tile([128, H], F32, tag="ald")
                # ald = cand0 * m + cand1 * (1 - m)
                c0 = spool.tile([128, H], F32, tag="c0")
                nc.vector.tensor_scalar(out=c0[:], in0=aldc[:, 0:H],
                                        scalar1=msf[:, b:b + 1], scalar2=None,
                                        op0=mybir.AluOpType.mult)
                c1 = spool.tile([128, H], F32, tag="c1")
                nc.vector.tensor_scalar(out=c1[:], in0=aldc[:, H:2 * H],
                                        scalar1=msf[:, b:b + 1], scalar2=None,
                                        op0=mybir.AluOpType.mult)
                nc.vector.tensor_tensor(out=c1[:], in0=aldc[:, H:2 * H], in1=c1[:],
                                        op=mybir.AluOpType.subtract)
                nc.vector.tensor_tensor(out=ald[:], in0=c0[:], in1=c1[:],
                                        op=mybir.AluOpType.add)

                # logits e = al_s + al_d (+ mask), lrelu, exp
                ee = spool.tile([128, k * H], F32, tag="ee")
                nc.vector.tensor_tensor(
                    out=ee[:].rearrange("p (k h) -> p k h", h=H),
                    in0=als[:].rearrange("p (k h) -> p k h", h=H),
                    in1=_bc(ald[:], 1, k),
                    op=mybir.AluOpType.add)
                nc.vector.tensor_tensor(
                    out=ee[:].rearrange("p (k h) -> p k h", h=H),
                    in0=ee[:].rearrange("p (k h) -> p k h", h=H),
                    in1=_bc(mk[:], 2, H),
                    op=mybir.AluOpType.add)
                ee2 = spool.tile([128, k * H], F32, tag="ee2")
                nc.vector.tensor_scalar_mul(ee2[:], ee[:], SLOPE)
                nc.vector.tensor_tensor(out=ee[:], in0=ee[:], in1=ee2[:],
                                        op=mybir.AluOpType.max)
                ex = spool.tile([128, k * H], BF16, tag="ex")
                den = spool.tile([128, H], F32, tag="den")
                ex3 = ex[:].rearrange("p (k h) -> p k h", h=H)
                ee3 = ee[:].rearrange("p (k h) -> p k h", h=H)
                for hh in range(H):
                    nc.scalar.activation(ex3[:, :, hh], ee3[:, :, hh],
                                         mybir.ActivationFunctionType.Exp,
                                         accum_out=den[:, hh:hh + 1])

                # weighted sum over slots
                wb = wpool.tile([128, k * D], BF16, tag="wb")
                ex_b = _bc(ex[:].rearrange("p (k h) -> p k h", h=H), 3, CH)
                nc.vector.tensor_tensor(
                    out=wb[:].rearrange("p (k h c) -> p k h c", h=H, c=CH),
                    in0=g[:].rearrange("p (k h c) -> p k h c", h=H, c=CH),
                    in1=ex_b,
                    op=mybir.AluOpType.mult)
                uu = spool.tile([128, D], F32, tag="uu")
                nc.vector.reduce_sum(
                    uu[:],
                    wb[:].rearrange("p (k d) -> p d k", d=D),
                    axis=mybir.AxisListType.X)

                # U / (den + 1e-16)
                rr_ = spool.tile([128, H], F32, tag="rr")
                nc.vector.tensor_scalar_add(rr_[:], den[:], 1e-16)
                nc.vector.reciprocal(rr_[:], rr_[:])
                nc.vector.tensor_tensor(
                    out=uu[:].rearrange("p (h c) -> p h c", h=H),
                    in0=uu[:].rearrange("p (h c) -> p h c", h=H),
                    in1=_bc(rr_[:], 2, CH),
                    op=mybir.AluOpType.mult)

                # transpose -> [ch, dst], head-mix, bias
                utp = ps2pool.tile([128, 128], F32, tag="utp", space="PSUM")
                nc.tensor.transpose(utp[:], uu[:], ident[:])
                uts = spool.tile([128, 128], F32, tag="uts")
                nc.vector.tensor_copy(uts[:], utp[:])
                otp = ps2pool.tile([128, 128], F32, tag="otp", space="PSUM")
                nc.tensor.matmul(otp[:], lhsT=mm[:], rhs=uts[:],
                                 start=True, stop=True)
                ots = spool.tile([128, 128], F32, tag="ots")
                nc.vector.tensor_scalar(out=ots[:], in0=otp[:], scalar1=bi[:],
                                        scalar2=None, op0=mybir.AluOpType.add)
                if b == NBLK - 1:
                    nc.vector.tensor_tensor(out=ots[:], in0=ots[:], in1=mft[:],
                                            op=mybir.AluOpType.mult)

                # partials
                sq = spool.tile([128, 128], F32, tag="sq")
                nc.vector.tensor_tensor(out=sq[:], in0=ots[:], in1=ots[:],
                                        op=mybir.AluOpType.mult)
                rs = spool.tile([128, 2], F32, tag="rs")
                nc.vector.reduce_sum(rs[:, 0:1], ots[:], axis=mybir.AxisListType.X)
                nc.vector.reduce_sum(rs[:, 1:2], sq[:], axis=mybir.AxisListType.X)
                nc.vector.tensor_tensor(out=pacc[:], in0=pacc[:], in1=rs[:],
                                        op=mybir.AluOpType.add)

                ob = spool.tile([128, 128], BF16, tag="ob")
                nc.vector.tensor_copy(ob[:], ots[:])
                nc.sync.dma_start(outb.ap()[:, b * 128:(b + 1) * 128], ob[:])
                nc.sync.dma_start(outf.ap()[:, b * 128:(b + 1) * 128], ots[:])

            nc.sync.dma_start(parts.ap(), pacc[:])

    # align each gather's SWDGE queue with its Tile-assigned DMASW sem lane
    for bb in nc.main_func.blocks:
        for ins in bb.instructions:
            if isinstance(ins, mybir.InstDMAGatherAnt):
                nm = _IDX_TO_PROC.get(ins.bass_scheduled_proc, "")
                if nm.startswith("DMASW"):
                    ins.queue_num = int(nm[5:]) % NQ

    nc.compile()
    return nc


# ----------------------------------------------------------------- driver

_TRACE = bool(os.environ.get("KERNEL_TRACE"))
LAST_EXEC_NS = []


def kernel(x, edge_index, W0, a_src0, a_dst0, b0, g0, be0,
           W1, a_src1, a_dst1, b1, g1, be1,
           W2, a_src2, a_dst2, b2):
    global LAST_EXEC_NS
    LAST_EXEC_NS = []
    prep = preprocess(np.asarray(edge_index))
    new_id = prep["new_id"]

    nc = build(prep["KLO"], prep["KHI"])

    xp = np.zeros((NPAD, IN), np.float32)
    xp[new_id] = np.asarray(x, np.float32)

    def rep_rows(v):
        return np.tile(np.asarray(v, np.float32).reshape(1, -1), (128, 1))

    eye = np.eye(128, dtype=np.float32)
    mix2 = np.zeros((128, 128), np.float32)
    mix2[0:64, 0:64] = 0.5 * np.eye(64)
    mix2[64:128, 0:64] = 0.5 * np.eye(64)
    maskf = np.ones((128, 128), np.float32)
    maskf[:, REAL_PER_CORE - 48 * 128:] = 0.0  # rows 106.. of last block

    layers = [
        dict(W=W0, a_src=a_src0, a_dst=a_dst0, bias=np.asarray(b0),
             g=np.full(128, np.sqrt(EPS), np.float32), be=np.zeros(128, np.float32),
             s=1.0, mix=eye),
        dict(W=W1, a_src=a_src1, a_dst=a_dst1, bias=np.asarray(b1),
             g=np.asarray(g0), be=np.asarray(be0), s=0.0, mix=eye),
        dict(W=W2, a_src=a_src2, a_dst=a_dst2, bias=np.concatenate(
            [np.asarray(b2), np.zeros(64, np.float32)]),
             g=np.asarray(g1), be=np.asarray(be1), s=0.0, mix=mix2),
    ]

    xt_cur = np.ascontiguousarray(xp.T).astype(np.float32)  # [128, NPAD]
    part_cur = np.zeros((128, 16), np.float32)

    outf = None
    for li, L in enumerate(layers):
        a_flat_s = np.asarray(L["a_src"], np.float32).reshape(-1)  # [128]
        a_flat_d = np.asarray(L["a_dst"], np.float32).reshape(-1)
        in_maps = []
        for c in range(N_CORES):
            in_maps.append(dict(
                xt=xt_cur.astype(np.dtype("bfloat16"))
                if hasattr(np, "bfloat16") else xt_cur,
                part=part_cur,
                gvec=np.asarray(L["g"], np.float32).reshape(128, 1),
                bevec=np.asarray(L["be"], np.float32).reshape(128, 1),
                srel=np.full((128, 1), L["s"], np.float32),
                wmat=np.asarray(L["W"], np.float32),
                asrct=rep_rows(a_flat_s),
                adstt=rep_rows(a_flat_d),
                mmat=np.asarray(L["mix"], np.float32),
                biasv=np.asarray(L["bias"], np.float32).reshape(128, 1),
                maskf=maskf,
                mself=prep["mself"][c],
                glo=prep["glo"][c],
                ghi=prep["ghi"][c],
                gmask=prep["gmask"][c],
            ))
        # dtype fixups (bf16 via ml_dtypes)
        import ml_dtypes
        for m in in_maps:
            m["xt"] = xt_cur.astype(ml_dtypes.bfloat16)
            m["wmat"] = m["wmat"].astype(ml_dtypes.bfloat16)
            m["asrct"] = m["asrct"].astype(ml_dtypes.bfloat16)
            m["adstt"] = m["adstt"].astype(ml_dtypes.bfloat16)

        res = bass_utils.run_bass_kernel_spmd(
            nc, in_maps, core_ids=list(range(N_CORES)), trace=_TRACE)
        if _TRACE and res.exec_time_ns:
            LAST_EXEC_NS.append(res.exec_time_ns)

        xt_cur = np.concatenate(
            [np.asarray(res.results[c]["outb"], np.float32)
             for c in range(N_CORES)], axis=1)
        part_cur = np.concatenate(
            [np.asarray(res.results[c]["parts"]) for c in range(N_CORES)],
            axis=1).reshape(128, 16)
        # reorder to [sums(8) | sumsq(8)]
        part_cur = np.concatenate(
            [part_cur[:, 0::2], part_cur[:, 1::2]], axis=1)
        if li == 2:
            outf = [np.asarray(res.results[c]["outf"]) for c in range(N_CORES)]

    full = np.concatenate(outf, axis=1)  # [128, NPAD]
    out = np.zeros((N, OUT), np.float32)
    real_mask = new_id  # new_id[orig] = padded position
    out[np.arange(N)] = full[:OUT, :].T[new_id]
    return out
